# revision 24
# baseline (speedup 1.0000x reference)
"""BrainModel kernel for 8 TRN2 NeuronCores (raw bass, no Tile).

Reference computation:
    gathered = x[:, idx]                              # [B, O, C]
    pre = einsum('boc,oc->bo', gathered, w_sparse) + b_sparse
    new_x = sigmoid(pre)                              # [B, O]
    q = new_x[:, -N_MOTORS:] @ w_motor.T + b_motor    # [B, A]

Only the last N_MOTORS=256 rows of idx/w_sparse/b_sparse reach q, so the
other 98720 output neurons are dead code. We shard those 256 motor
neurons across the 8 cores (32 each -> 1024 gathered x-columns per core).

The gather is 8 SWDGE indirect DMAs of 128 descriptors each (measured HW
facts: the indirect path consumes exactly ONE index per partition per
instruction - multi-column offset APs are ignored/garbage, dma_gather's
Q7 ucode is 30x slower per descriptor plus a 9us library load, and HWDGE
engines crash on indirect DMAs - so 8 x ~1.1us serialized emission on
the Pool sequencer is the floor).

Over the original fp32 baseline (~28.5-29.8us -> ~26.3us):
  - x table in bf16 [100000, 64]: gather descriptors move 128B instead
    of 256B, and all matmuls run single-pass bf16 (the fp32 path pumps
    every matmul twice as LOW/HIGH). rel err ~3e-3, tolerance is 2e-2.
  - a warmup gather (row 0 x 128 into scratch) keeps the Q7 SWDGE busy
    from preamble exit until the index tile lands, eating the ~0.9us
    SWDGE cold-start inside the idx-DMA wait window.
  - contiguous standalone idx tensor, DMA'd by Sync the instant the
    preamble barrier releases (Sync exits it first).
  - the Bass-init const-AP memsets are stripped from the Pool stream
    (nothing here reads them; they sat before the preamble barrier).
  - sigmoid outputs bf16 so the motor matmul is also single-pass.
Host sums the 8 partials and transposes to [B, A].
"""

from contextlib import ExitStack

import ml_dtypes
import numpy as np

import concourse.bass as bass
from concourse import mybir

N_NEURONS = 100000
N_MOTORS = 256
N_CONN = 32
N_ACT = 16
BATCH = 64
N_CORES = 8
M_PER_CORE = N_MOTORS // N_CORES  # 32 motor neurons per core
R = M_PER_CORE * N_CONN  # 1024 gathered x-rows per core
P = 128  # SBUF partitions
J = R // P  # 8 gather/matmul chunks

C_WK = J * M_PER_CORE // 2  # 128: f32 cols of bf16 Wk
C_WMT = C_WK + N_ACT // 2  # 136: wmT end (bf16 [32, 16])
C_BS = C_WMT  # 136: b_sparse col
C_BM = C_BS + 1  # 137: b_motor/8 col
C_IDX = C_BM + 1  # 138: idx cols (8 x int32)
AUXC = C_IDX + J  # 146

BF16 = ml_dtypes.bfloat16

_CACHE: dict = {}


def _build_nc() -> bass.Bass:
    f32 = mybir.dt.float32
    bf16 = mybir.dt.bfloat16
    i32 = mybir.dt.int32
    nc = bass.Bass(enable_partition_id=False, monotonic_sem_count=0)

    # Drop the init-emitted const-AP memsets from the Pool stream: they sit on
    # the preamble's critical path (~0.4us before the barrier releases) and
    # nothing in this kernel reads the const tiles (all-bf16 matmuls, explicit
    # AP biases, no bounds checks).
    blk = nc.m.functions[0].blocks[0]
    blk.instructions = [i for i in blk.instructions if i.opcode != "Memset"]

    tbl = nc.declare_dram_parameter("tbl", [N_NEURONS, BATCH], bf16, isOutput=False)
    aux1 = nc.declare_dram_parameter("aux1", [P, J], f32, isOutput=False)
    aux2 = nc.declare_dram_parameter("aux2", [P, C_IDX], f32, isOutput=False)
    out = nc.declare_dram_parameter("out", [N_ACT, BATCH], f32, isOutput=True)

    with ExitStack() as ctx:
        aux1_sb = ctx.enter_context(nc.sbuf_tensor("aux1_sb", [P, J], f32))
        aux_sb = ctx.enter_context(nc.sbuf_tensor("aux_sb", [P, C_IDX], f32))
        G = ctx.enter_context(nc.sbuf_tensor("G", [P, J * BATCH], bf16))
        wscr = ctx.enter_context(nc.sbuf_tensor("wscr", [P, BATCH], bf16))
        widx = ctx.enter_context(nc.sbuf_tensor("widx", [P, 1], i32))
        s_sb = ctx.enter_context(nc.sbuf_tensor("s_sb", [M_PER_CORE, BATCH], bf16))
        q_sb = ctx.enter_context(nc.sbuf_tensor("q_sb", [N_ACT, BATCH], f32))
        pre_ps = ctx.enter_context(nc.psum_tensor("pre_ps", [M_PER_CORE, BATCH], f32))
        q_ps = ctx.enter_context(nc.psum_tensor("q_ps", [N_ACT, BATCH], f32))
        isem = ctx.enter_context(nc.semaphore("isem"))
        wsem = ctx.enter_context(nc.semaphore("wsem"))
        wgsem = ctx.enter_context(nc.semaphore("wgsem"))
        odma_sem = ctx.enter_context(nc.semaphore("odma_sem"))
        # One completion sem per gather chunk: a single shared sem would be
        # racy -- each DMA's 16 increments come from 16 independent SDMA
        # engines, so a running count can reach 16*(j+1) before chunk j has
        # fully landed.
        gdma_sems = [
            ctx.enter_context(nc.semaphore(f"gdma_sem{j}")) for j in range(J)
        ]
        pe_sem = ctx.enter_context(nc.semaphore("pe_sem"))
        act_sem = ctx.enter_context(nc.semaphore("act_sem"))
        warm_sb = ctx.enter_context(nc.sbuf_tensor("warm_sb", [1, 1], f32))
        block = ctx.enter_context(nc.Block())

        @block.sync
        def _(sync):
            # idx first (small, contiguous) so the gathers start ASAP; weights
            # on their own sem (completion order of two DMAs is unordered).
            sync.dma_start(out=aux1_sb[:], in_=aux1[:]).then_inc(isem, 16)
            sync.dma_start(out=aux_sb[:], in_=aux2[:]).then_inc(wsem, 16)
            sync.wait_ge(odma_sem, 16)

        @block.gpsimd
        def _(gpsimd):
            # Warmup gather (row 0 into scratch): eats the SWDGE cold-start
            # stall inside the idx-DMA wait window, ending just before the
            # idx tile lands so the first real gather dispatches hot.
            gpsimd.memset(widx[:], 0)
            gpsimd.indirect_dma_start(
                out=wscr[:],
                out_offset=None,
                in_=tbl[:],
                in_offset=bass.IndirectOffsetOnAxis(ap=widx[:], axis=0),
            ).then_inc(wgsem, 16)
            gpsimd.wait_ge(isem, 16)
            # One index per partition per instruction: partition p of the dest
            # gets dest-free-size contiguous bytes starting at tbl row idx[p].
            for j in range(J):
                gpsimd.indirect_dma_start(
                    out=G[:, j * BATCH : (j + 1) * BATCH],
                    out_offset=None,
                    in_=tbl[:],
                    in_offset=bass.IndirectOffsetOnAxis(
                        ap=aux1_sb[:, j : j + 1].bitcast(i32),
                        axis=0,
                    ),
                ).then_inc(gdma_sems[j], 16)

        @block.tensor
        def _(tensor):
            tensor.wait_ge(wsem, 16)
            wk = aux_sb[:, :C_WK].bitcast(mybir.dt.bfloat16)  # [128, J*32]
            # pre[m, b] = sum_{p,j} Wk[p, j*32+m] * x[b, idx_flat[p*J+j]]
            for j in range(J):
                tensor.wait_ge(gdma_sems[j], 16)
                mm = tensor.matmul(
                    pre_ps[:],
                    wk[:, j * M_PER_CORE : (j + 1) * M_PER_CORE],
                    G[:, j * BATCH : (j + 1) * BATCH],
                    start=(j == 0),
                    stop=(j == J - 1),
                )
            mm.then_inc(pe_sem, 1)
            tensor.wait_ge(act_sem, 1)
            # q_part[a, b] = sum_m wmT[m, a] * s[m, b]
            tensor.matmul(
                q_ps[:],
                aux_sb[:M_PER_CORE, C_WK:C_WMT].bitcast(mybir.dt.bfloat16),
                s_sb[:],
                start=True,
                stop=True,
            ).then_inc(pe_sem, 1)

        @block.scalar
        def _(scalar):
            # Dummy activation preloads the sigmoid LUT off the critical path
            # (the table load is ~1.3us and otherwise serializes after the
            # last matmul). Reads its own scratch; the value is irrelevant.
            scalar.activation(
                warm_sb[:],
                warm_sb[:],
                mybir.ActivationFunctionType.Sigmoid,
                bias=warm_sb[:],  # explicit AP: avoids the const-0 tile
            )
            scalar.wait_ge(pe_sem, 1)
            # s = sigmoid(pre + b_sparse), bf16 out for the 1-pass motor matmul
            scalar.activation(
                s_sb[:],
                pre_ps[:],
                mybir.ActivationFunctionType.Sigmoid,
                bias=aux_sb[:M_PER_CORE, C_BS : C_BS + 1],
            ).then_inc(act_sem, 1)
            scalar.wait_ge(pe_sem, 2)
            # q_sb = q_ps + b_motor/8 (PSUM -> SBUF)
            scalar.activation(
                q_sb[:],
                q_ps[:],
                mybir.ActivationFunctionType.Identity,
                bias=aux_sb[:N_ACT, C_BM : C_BM + 1],
            )
            # ScalarE is HWDGE-capable: issue the output DMA right here,
            # skipping a cross-engine semaphore hop to Sync.
            scalar.dma_start(out=out[:], in_=q_sb[:]).then_inc(odma_sem, 16)

    # Pack each SDMA engine's gather descriptors into one packet: our 128B
    # descriptors are the size class where packet concat amortizes per-packet
    # overhead (indirect_dma_start doesn't expose the flag).
    for b in nc.m.functions[0].blocks:
        for inst in b.instructions:
            if inst.opcode == "DMACopy" and inst.queue == "qPoolDynamic":
                inst.single_packet = True

    return nc


def _get_nc() -> bass.Bass:
    if "nc" not in _CACHE:
        _CACHE["nc"] = _build_nc()
    return _CACHE["nc"]


def make_in_maps(x, idx, w_sparse, b_sparse, w_motor, b_motor):
    """Shard FULL inputs into the 8 per-core input dicts."""
    x = np.asarray(x, dtype=np.float32)
    idx_m = np.asarray(idx)[-N_MOTORS:].astype(np.int32)  # [256, 32]
    w_m = np.asarray(w_sparse, dtype=np.float32)[-N_MOTORS:]  # [256, 32]
    b_m = np.asarray(b_sparse, dtype=np.float32)[-N_MOTORS:]  # [256]
    wm = np.asarray(w_motor, dtype=np.float32)  # [16, 256]
    bm = np.asarray(b_motor, dtype=np.float32)  # [16]

    tbl = np.ascontiguousarray(x.T.astype(BF16))  # [N_NEURONS, B], row n = x[:, n]

    # flat position r = p*J + j: gathered row lands at G[p, j*B:(j+1)*B]; its
    # weight multiplies motor column m = r//32 of matmul chunk j.
    r = np.arange(R)
    o_l, c = r // N_CONN, r % N_CONN
    p_r, j_r = r // J, r % J

    in_maps = []
    for k in range(N_CORES):
        rows = slice(k * M_PER_CORE, (k + 1) * M_PER_CORE)
        w_core = w_m[rows]  # [32, 32]

        aux1 = np.ascontiguousarray(idx_m[rows].reshape(P, J)).view(np.float32)

        aux2 = np.zeros((P, C_IDX), np.float32)
        wk = np.zeros((P, J * M_PER_CORE), BF16)
        wk[p_r, j_r * M_PER_CORE + o_l] = w_core[o_l, c].astype(BF16)
        aux2[:, :C_WK] = wk.view(np.float32)
        aux2[:M_PER_CORE, C_WK:C_WMT] = (
            np.ascontiguousarray(wm[:, rows].T.astype(BF16))
            .view(np.float32)
            .reshape(M_PER_CORE, N_ACT // 2)
        )
        aux2[:M_PER_CORE, C_BS] = b_m[rows]
        aux2[:N_ACT, C_BM] = bm / N_CORES

        in_maps.append({"tbl": tbl, "aux1": aux1, "aux2": aux2})
    return in_maps


def combine_outputs(partials):
    """Reduce the 8 per-core [A, B] partials to the full [B, A] output."""
    q = np.sum(np.stack(partials, axis=0), axis=0, dtype=np.float64)
    return np.ascontiguousarray(q.T).astype(np.float32)


def _ensure_trace_hook_importable():
    """bass_utils' axon trace path imports antenv.axon_hooks; some containers
    ship an antenv without it. Provide a null hook so trace degrades to a
    plain run instead of crashing."""
    import os

    if not os.environ.get("BASS_TRACE"):
        return
    try:
        import antenv.axon_hooks  # noqa: F401
    except ImportError:
        import sys
        import types

        import antenv

        m = types.ModuleType("antenv.axon_hooks")
        state = {"hook": None}
        m.set_axon_ntff_profile_hook = lambda h: state.__setitem__("hook", h)
        m.get_axon_ntff_profile_hook = lambda: state["hook"]
        sys.modules["antenv.axon_hooks"] = m
        antenv.axon_hooks = m


def kernel(x, idx, w_sparse, b_sparse, w_motor, b_motor):
    from concourse.bass_utils import run_bass_kernel_spmd

    _ensure_trace_hook_importable()
    nc = _get_nc()
    in_maps = make_in_maps(x, idx, w_sparse, b_sparse, w_motor, b_motor)
    res = run_bass_kernel_spmd(nc, in_maps, core_ids=list(range(N_CORES)))
    _CACHE["last_results"] = res
    return combine_outputs([res.results[k]["out"] for k in range(N_CORES)])


# revision 25
# speedup vs baseline: 1.0104x; 1.0104x over previous
"""BrainModel kernel for 8 TRN2 NeuronCores (raw bass, no Tile).

Reference computation:
    gathered = x[:, idx]                              # [B, O, C]
    pre = einsum('boc,oc->bo', gathered, w_sparse) + b_sparse
    new_x = sigmoid(pre)                              # [B, O]
    q = new_x[:, -N_MOTORS:] @ w_motor.T + b_motor    # [B, A]

Only the last N_MOTORS=256 rows of idx/w_sparse/b_sparse reach q, so the
other 98720 output neurons are dead code. We shard those 256 motor
neurons across the 8 cores (32 each -> 1024 gathered x-columns per core).

The gather is 8 SWDGE indirect DMAs of 128 descriptors each (measured HW
facts: the indirect path consumes exactly ONE index per partition per
instruction - multi-column offset APs are ignored/garbage, dma_gather's
Q7 ucode is 30x slower per descriptor plus a 9us library load, and HWDGE
engines crash on indirect DMAs - so 8 x ~1.1us serialized emission on
the Pool sequencer is the floor).

Over the original fp32 baseline (~28.5-29.8us -> ~26.3us):
  - x table in bf16 [100000, 64]: gather descriptors move 128B instead
    of 256B, and all matmuls run single-pass bf16 (the fp32 path pumps
    every matmul twice as LOW/HIGH). rel err ~3e-3, tolerance is 2e-2.
  - a warmup gather (row 0 x 128 into scratch) keeps the Q7 SWDGE busy
    from preamble exit until the index tile lands, eating the ~0.9us
    SWDGE cold-start inside the idx-DMA wait window.
  - contiguous standalone idx tensor, DMA'd by Sync the instant the
    preamble barrier releases (Sync exits it first).
  - the Bass-init const-AP memsets are stripped from the Pool stream
    (nothing here reads them; they sat before the preamble barrier).
  - sigmoid outputs bf16 so the motor matmul is also single-pass.
Host sums the 8 partials and transposes to [B, A].
"""

from contextlib import ExitStack

import ml_dtypes
import numpy as np

import concourse.bass as bass
from concourse import mybir

N_NEURONS = 100000
N_MOTORS = 256
N_CONN = 32
N_ACT = 16
BATCH = 64
N_CORES = 8
M_PER_CORE = N_MOTORS // N_CORES  # 32 motor neurons per core
R = M_PER_CORE * N_CONN  # 1024 gathered x-rows per core
P = 128  # SBUF partitions
J = R // P  # 8 gather/matmul chunks

C_WK = J * M_PER_CORE // 2  # 128: f32 cols of bf16 Wk
C_WMT = C_WK + N_ACT // 2  # 136: wmT end (bf16 [32, 16])
C_BS = C_WMT  # 136: b_sparse col
C_BM = C_BS + 1  # 137: b_motor/8 col
C_IDX = C_BM + 1  # 138: idx cols (8 x int32)
AUXC = C_IDX + J  # 146

BF16 = ml_dtypes.bfloat16

_CACHE: dict = {}


def _build_nc() -> bass.Bass:
    f32 = mybir.dt.float32
    bf16 = mybir.dt.bfloat16
    i32 = mybir.dt.int32
    nc = bass.Bass(enable_partition_id=False, monotonic_sem_count=0)

    # Drop the init-emitted const-AP memsets from the Pool stream: they sit on
    # the preamble's critical path (~0.4us before the barrier releases) and
    # nothing in this kernel reads the const tiles (all-bf16 matmuls, explicit
    # AP biases, no bounds checks).
    blk = nc.m.functions[0].blocks[0]
    blk.instructions = [i for i in blk.instructions if i.opcode != "Memset"]

    tbl = nc.declare_dram_parameter("tbl", [N_NEURONS, BATCH], bf16, isOutput=False)
    aux1 = nc.declare_dram_parameter("aux1", [P, J], f32, isOutput=False)
    aux2 = nc.declare_dram_parameter("aux2", [P, C_IDX], f32, isOutput=False)
    out = nc.declare_dram_parameter("out", [N_ACT, BATCH], f32, isOutput=True)

    with ExitStack() as ctx:
        aux1_sb = ctx.enter_context(nc.sbuf_tensor("aux1_sb", [P, J], f32))
        aux_sb = ctx.enter_context(nc.sbuf_tensor("aux_sb", [P, C_IDX], f32))
        G = ctx.enter_context(nc.sbuf_tensor("G", [P, J * BATCH], bf16))
        wscr = ctx.enter_context(nc.sbuf_tensor("wscr", [P, BATCH], bf16))
        widx = ctx.enter_context(nc.sbuf_tensor("widx", [P, 1], i32))
        s_sb = ctx.enter_context(nc.sbuf_tensor("s_sb", [M_PER_CORE, BATCH], bf16))
        q_sb = ctx.enter_context(nc.sbuf_tensor("q_sb", [N_ACT, BATCH], f32))
        pre_ps = ctx.enter_context(nc.psum_tensor("pre_ps", [M_PER_CORE, BATCH], f32))
        q_ps = ctx.enter_context(nc.psum_tensor("q_ps", [N_ACT, BATCH], f32))
        isem = ctx.enter_context(nc.semaphore("isem"))
        wsem = ctx.enter_context(nc.semaphore("wsem"))
        wgsem = ctx.enter_context(nc.semaphore("wgsem"))
        odma_sem = ctx.enter_context(nc.semaphore("odma_sem"))
        # One completion sem per gather chunk: a single shared sem would be
        # racy -- each DMA's 16 increments come from 16 independent SDMA
        # engines, so a running count can reach 16*(j+1) before chunk j has
        # fully landed.
        gdma_sems = [
            ctx.enter_context(nc.semaphore(f"gdma_sem{j}")) for j in range(J)
        ]
        pe_sem = ctx.enter_context(nc.semaphore("pe_sem"))
        act_sem = ctx.enter_context(nc.semaphore("act_sem"))
        warm_sb = ctx.enter_context(nc.sbuf_tensor("warm_sb", [1, 1], f32))
        block = ctx.enter_context(nc.Block())

        @block.sync
        def _(sync):
            # idx first (small, contiguous) so the gathers start ASAP; weights
            # on their own sem (completion order of two DMAs is unordered).
            sync.dma_start(out=aux1_sb[:], in_=aux1[:]).then_inc(isem, 16)
            sync.dma_start(out=aux_sb[:], in_=aux2[:]).then_inc(wsem, 16)
            sync.wait_ge(odma_sem, 16)

        @block.gpsimd
        def _(gpsimd):
            # Warmup gather (row 0 into scratch): eats the SWDGE cold-start
            # stall inside the idx-DMA wait window, ending just before the
            # idx tile lands so the first real gather dispatches hot.
            gpsimd.memset(widx[:], 0)
            gpsimd.indirect_dma_start(
                out=wscr[:],
                out_offset=None,
                in_=tbl[:],
                in_offset=bass.IndirectOffsetOnAxis(ap=widx[:], axis=0),
            ).then_inc(wgsem, 16)
            # A few cheap ops before the wait so isem is stale-satisfied when
            # the wait is processed: a freshly-satisfied wait costs the next
            # SWDGE dispatch ~0.8us (sem arm/disarm slow path), a stale one
            # ~0.2us (measured).
            for _s in range(4):
                gpsimd.memset(widx[:1, :], 0)
            gpsimd.wait_ge(isem, 16)
            # One index per partition per instruction: partition p of the dest
            # gets dest-free-size contiguous bytes starting at tbl row idx[p].
            for j in range(J):
                gpsimd.indirect_dma_start(
                    out=G[:, j * BATCH : (j + 1) * BATCH],
                    out_offset=None,
                    in_=tbl[:],
                    in_offset=bass.IndirectOffsetOnAxis(
                        ap=aux1_sb[:, j : j + 1].bitcast(i32),
                        axis=0,
                    ),
                ).then_inc(gdma_sems[j], 16)

        @block.tensor
        def _(tensor):
            tensor.wait_ge(wsem, 16)
            wk = aux_sb[:, :C_WK].bitcast(mybir.dt.bfloat16)  # [128, J*32]
            # pre[m, b] = sum_{p,j} Wk[p, j*32+m] * x[b, idx_flat[p*J+j]]
            for j in range(J):
                tensor.wait_ge(gdma_sems[j], 16)
                mm = tensor.matmul(
                    pre_ps[:],
                    wk[:, j * M_PER_CORE : (j + 1) * M_PER_CORE],
                    G[:, j * BATCH : (j + 1) * BATCH],
                    start=(j == 0),
                    stop=(j == J - 1),
                )
            mm.then_inc(pe_sem, 1)
            tensor.wait_ge(act_sem, 1)
            # q_part[a, b] = sum_m wmT[m, a] * s[m, b]
            tensor.matmul(
                q_ps[:],
                aux_sb[:M_PER_CORE, C_WK:C_WMT].bitcast(mybir.dt.bfloat16),
                s_sb[:],
                start=True,
                stop=True,
            ).then_inc(pe_sem, 1)

        @block.scalar
        def _(scalar):
            # Dummy activation preloads the sigmoid LUT off the critical path
            # (the table load is ~1.3us and otherwise serializes after the
            # last matmul). Reads its own scratch; the value is irrelevant.
            scalar.activation(
                warm_sb[:],
                warm_sb[:],
                mybir.ActivationFunctionType.Sigmoid,
                bias=warm_sb[:],  # explicit AP: avoids the const-0 tile
            )
            scalar.wait_ge(pe_sem, 1)
            # s = sigmoid(pre + b_sparse), bf16 out for the 1-pass motor matmul
            scalar.activation(
                s_sb[:],
                pre_ps[:],
                mybir.ActivationFunctionType.Sigmoid,
                bias=aux_sb[:M_PER_CORE, C_BS : C_BS + 1],
            ).then_inc(act_sem, 1)
            scalar.wait_ge(pe_sem, 2)
            # q_sb = q_ps + b_motor/8 (PSUM -> SBUF)
            scalar.activation(
                q_sb[:],
                q_ps[:],
                mybir.ActivationFunctionType.Identity,
                bias=aux_sb[:N_ACT, C_BM : C_BM + 1],
            )
            # ScalarE is HWDGE-capable: issue the output DMA right here,
            # skipping a cross-engine semaphore hop to Sync.
            scalar.dma_start(out=out[:], in_=q_sb[:]).then_inc(odma_sem, 16)

    # Pack each SDMA engine's gather descriptors into one packet: our 128B
    # descriptors are the size class where packet concat amortizes per-packet
    # overhead (indirect_dma_start doesn't expose the flag).
    for b in nc.m.functions[0].blocks:
        for inst in b.instructions:
            if inst.opcode == "DMACopy" and inst.queue == "qPoolDynamic":
                inst.single_packet = True

    return nc


def _get_nc() -> bass.Bass:
    if "nc" not in _CACHE:
        _CACHE["nc"] = _build_nc()
    return _CACHE["nc"]


def make_in_maps(x, idx, w_sparse, b_sparse, w_motor, b_motor):
    """Shard FULL inputs into the 8 per-core input dicts."""
    x = np.asarray(x, dtype=np.float32)
    idx_m = np.asarray(idx)[-N_MOTORS:].astype(np.int32)  # [256, 32]
    w_m = np.asarray(w_sparse, dtype=np.float32)[-N_MOTORS:]  # [256, 32]
    b_m = np.asarray(b_sparse, dtype=np.float32)[-N_MOTORS:]  # [256]
    wm = np.asarray(w_motor, dtype=np.float32)  # [16, 256]
    bm = np.asarray(b_motor, dtype=np.float32)  # [16]

    tbl = np.ascontiguousarray(x.T.astype(BF16))  # [N_NEURONS, B], row n = x[:, n]

    # flat position r = p*J + j: gathered row lands at G[p, j*B:(j+1)*B]; its
    # weight multiplies motor column m = r//32 of matmul chunk j.
    r = np.arange(R)
    o_l, c = r // N_CONN, r % N_CONN
    p_r, j_r = r // J, r % J

    in_maps = []
    for k in range(N_CORES):
        rows = slice(k * M_PER_CORE, (k + 1) * M_PER_CORE)
        w_core = w_m[rows]  # [32, 32]

        aux1 = np.ascontiguousarray(idx_m[rows].reshape(P, J)).view(np.float32)

        aux2 = np.zeros((P, C_IDX), np.float32)
        wk = np.zeros((P, J * M_PER_CORE), BF16)
        wk[p_r, j_r * M_PER_CORE + o_l] = w_core[o_l, c].astype(BF16)
        aux2[:, :C_WK] = wk.view(np.float32)
        aux2[:M_PER_CORE, C_WK:C_WMT] = (
            np.ascontiguousarray(wm[:, rows].T.astype(BF16))
            .view(np.float32)
            .reshape(M_PER_CORE, N_ACT // 2)
        )
        aux2[:M_PER_CORE, C_BS] = b_m[rows]
        aux2[:N_ACT, C_BM] = bm / N_CORES

        in_maps.append({"tbl": tbl, "aux1": aux1, "aux2": aux2})
    return in_maps


def combine_outputs(partials):
    """Reduce the 8 per-core [A, B] partials to the full [B, A] output."""
    q = np.sum(np.stack(partials, axis=0), axis=0, dtype=np.float64)
    return np.ascontiguousarray(q.T).astype(np.float32)


def _ensure_trace_hook_importable():
    """bass_utils' axon trace path imports antenv.axon_hooks; some containers
    ship an antenv without it. Provide a null hook so trace degrades to a
    plain run instead of crashing."""
    import os

    if not os.environ.get("BASS_TRACE"):
        return
    try:
        import antenv.axon_hooks  # noqa: F401
    except ImportError:
        import sys
        import types

        import antenv

        m = types.ModuleType("antenv.axon_hooks")
        state = {"hook": None}
        m.set_axon_ntff_profile_hook = lambda h: state.__setitem__("hook", h)
        m.get_axon_ntff_profile_hook = lambda: state["hook"]
        sys.modules["antenv.axon_hooks"] = m
        antenv.axon_hooks = m


def kernel(x, idx, w_sparse, b_sparse, w_motor, b_motor):
    from concourse.bass_utils import run_bass_kernel_spmd

    _ensure_trace_hook_importable()
    nc = _get_nc()
    in_maps = make_in_maps(x, idx, w_sparse, b_sparse, w_motor, b_motor)
    res = run_bass_kernel_spmd(nc, in_maps, core_ids=list(range(N_CORES)))
    _CACHE["last_results"] = res
    return combine_outputs([res.results[k]["out"] for k in range(N_CORES)])
